# revision 13
# baseline (speedup 1.0000x reference)
"""Behler-Parrinello NN on Trainium2: 8-core data-parallel Bass/Tile kernel.

Strategy
--------
- Shard the atom axis N across 8 cores (each core: 4 types x 16384 atoms).
- Host pre-transposes x to [T, F, Nshard] so features land on SBUF partitions;
  the whole MLP then runs on the PE with atoms on the moving/free axis
  (float32r = TF32 matmuls, N=512 per matmul -> full-rate 4-byte path).
- Biases b2/b3/b4 are folded into the weights via a ones-row appended to
  h1/h2/h3 (the contraction chunks have partition headroom: 125->126,
  100->101), so the post-matmul relus are bias-free single wide ops.
- Layer 4 ([1,100] @ h3) is computed as out = h3_chunk.T @ W4 per 128-atom
  group (fp16) so per-atom energies land with atoms on PSUM *partitions*.
- The scatter-add e[ind] += v is turned into matmuls: with m = q*128 + r,
  A[n,q] = v_n * (q_n == q), B[n,r] = (r_n == r)  =>  e_qr += A.T @ B  (fp16
  operands, f32 PSUM accumulation) over the entire kernel in one persistent
  PSUM tile; v lives in spare columns of the same PSUM bank.
- Host sums the 8 per-core partial e grids.
"""

import os
from contextlib import ExitStack

import numpy as np

import concourse.bacc as bacc
import concourse.mybir as mybir
import concourse.tile as tile
from concourse.bass_utils import run_bass_kernel_spmd

F32 = mybir.dt.float32
F32R = mybir.dt.float32r
F16 = mybir.dt.float16
AF = mybir.ActivationFunctionType
ALU = mybir.AluOpType

T, F = 4, 128
H1, H2, H3 = 500, 200, 100
MOLS = 16384
NCORES = 8
NFULL = 131072
NSHARD = NFULL // NCORES  # 16384 atoms per type per core
BLK = 512                 # atoms per block (PE moving-dim / PSUM bank)
GPB = BLK // 128          # 128-atom groups per block

H1CS = 125  # H1 = 4 x 125
H1C = 4
H2CS = 100  # H2 = 2 x 100 (M chunks for layer 2)
H2C = 2
H3KC = 2    # layer-3 contraction 200 = 2 x 100


def build_program(ns=NSHARD, t_types=T, debug=False):
    """Build and compile the single-core program (SPMD: same on all cores)."""
    assert ns % BLK == 0
    nblk = ns // BLK
    X = ns // 128

    nc = bacc.Bacc(
        "TRN2", target_bir_lowering=False, debug=False, enable_asserts=False
    )

    def din(name, shape, dt=F32):
        return nc.dram_tensor(name, shape, dt, kind="ExternalInput").ap()

    xT = din("xT", [t_types, F, ns])
    qT = din("qT", [t_types, 128, X])
    rT = din("rT", [t_types, 128, X])
    w1t = din("w1t", [t_types, F, H1])
    w2t = din("w2t", [t_types, H1CS + 1, H1C * H2])
    w3t = din("w3t", [t_types, H3 + 1, H3KC * H3])
    w4t = din("w4t", [t_types, H3 + 1, 1], F16)
    b1 = din("b1", [t_types, H1, 1])
    iota = din("iota", [128, BLK])
    ones32 = din("ones32", [1, H1C * BLK])
    ones16 = din("ones16", [1, BLK], F16)
    eout = nc.dram_tensor("e_part", [128, 128], F32, kind="ExternalOutput").ap()
    dbg = {}
    if debug:
        dbg["h1"] = nc.dram_tensor("d_h1", [H1CS + 1, H1C * BLK], F32,
                                   kind="ExternalOutput").ap()
        dbg["h2"] = nc.dram_tensor("d_h2", [H2CS + 1, H2C * BLK], F32,
                                   kind="ExternalOutput").ap()
        dbg["h3"] = nc.dram_tensor("d_h3", [H3 + 1, BLK], F16,
                                   kind="ExternalOutput").ap()
        dbg["v"] = nc.dram_tensor("d_v", [128, GPB], F32,
                                  kind="ExternalOutput").ap()
        dbg["t1"] = nc.dram_tensor("d_t1", [128, GPB * 128], F32,
                                   kind="ExternalOutput").ap()
        dbg["A"] = nc.dram_tensor("d_A", [128, GPB * 128], F16,
                                  kind="ExternalOutput").ap()
        dbg["B"] = nc.dram_tensor("d_B", [128, GPB * 128], F16,
                                  kind="ExternalOutput").ap()

    n_scatter = t_types * nblk * GPB
    scnt = 0
    pend = []

    with tile.TileContext(nc) as tc:
        with ExitStack() as ctx:
            const = ctx.enter_context(tc.tile_pool(name="const", bufs=1))
            wpool = ctx.enter_context(tc.tile_pool(name="w", bufs=2))
            xpool = ctx.enter_context(tc.tile_pool(name="x", bufs=4))
            hpool = ctx.enter_context(tc.tile_pool(name="h", bufs=1))
            abpool = ctx.enter_context(tc.tile_pool(name="ab", bufs=3))
            vsbp = ctx.enter_context(tc.tile_pool(name="vsb", bufs=3))
            z1pool = ctx.enter_context(tc.tile_pool(name="z1", bufs=1, space="PSUM"))
            z2pool = ctx.enter_context(tc.tile_pool(name="z2", bufs=1, space="PSUM"))
            vpool = ctx.enter_context(tc.tile_pool(name="v", bufs=1, space="PSUM"))
            epool = ctx.enter_context(tc.tile_pool(name="e", bufs=1, space="PSUM"))

            iota_sb = const.tile([128, BLK], F32, tag="iota")
            nc.sync.dma_start(iota_sb[:], iota[:])

            # persistent PSUM e accumulator (its bank sees no other matmul
            # groups: start=True clears the WHOLE bank, not just dst elements)
            e_ps = epool.tile([128, 128], F32, tag="eacc")

            # manual double buffers so the appended ones-row survives reuse
            h1b = [const.tile([H1CS + 1, H1C * BLK], F32R,
                              name=f"h1_{i}", tag=f"h1_{i}")
                   for i in range(2)]
            h2b = [const.tile([H2CS + 1, H2C * BLK], F32R,
                              name=f"h2_{i}", tag=f"h2_{i}")
                   for i in range(2)]
            h3b = [const.tile([H3 + 1, BLK], F16,
                              name=f"h3_{i}", tag=f"h3_{i}")
                   for i in range(2)]
            for i in range(2):
                nc.sync.dma_start(
                    h1b[i][H1CS:H1CS + 1, :], ones32[:].bitcast(F32R)
                )
                nc.sync.dma_start(
                    h2b[i][H2CS:H2CS + 1, :],
                    ones32[:, :H2C * BLK].bitcast(F32R),
                )
                nc.sync.dma_start(h3b[i][H3:H3 + 1, :], ones16[:])

            for t in range(t_types):
                w1_sb = wpool.tile([F, H1], F32R, tag="w1")
                nc.sync.dma_start(w1_sb[:], w1t[t].bitcast(F32R))
                w2_sb = wpool.tile([H1CS + 1, H1C * H2], F32R, tag="w2")
                nc.sync.dma_start(w2_sb[:], w2t[t].bitcast(F32R))
                w3_sb = wpool.tile([H3 + 1, H3KC * H3], F32R, tag="w3")
                nc.sync.dma_start(w3_sb[:], w3t[t].bitcast(F32R))
                w4_sb = wpool.tile([H3 + 1, 1], F16, tag="w4")
                nc.sync.dma_start(w4_sb[:], w4t[t])
                b1_sb = wpool.tile([H1CS, H1C], F32, tag="b1")
                nc.sync.dma_start(
                    b1_sb[:], b1[t].rearrange("(c p) x -> p (c x)", c=H1C)
                )
                q_sb = wpool.tile([128, X], F32, tag="q")
                nc.sync.dma_start(q_sb[:], qT[t])
                r_sb = wpool.tile([128, X], F32, tag="r")
                nc.sync.dma_start(r_sb[:], rT[t])

                for b in range(nblk):
                    h1 = h1b[b % 2]
                    h2 = h2b[b % 2]
                    h3 = h3b[b % 2]

                    xt = xpool.tile([128, BLK], F32R, tag="xt")
                    nc.sync.dma_start(
                        xt[:], xT[t, :, b * BLK:(b + 1) * BLK].bitcast(F32R)
                    )

                    # ---- layer 1: z1 = W1 @ x ; h1 = relu(z1 + b1) ----
                    z1 = z1pool.tile([H1CS, H1C * BLK], F32, tag="z1")
                    for c in range(H1C):
                        nc.tensor.matmul(
                            z1[:, c * BLK:(c + 1) * BLK],
                            lhsT=w1_sb[:, c * H1CS:(c + 1) * H1CS],
                            rhs=xt[:],
                            start=True,
                            stop=True,
                        )
                    for c in range(H1C):
                        dst = h1[:H1CS, c * BLK:(c + 1) * BLK]
                        src = z1[:, c * BLK:(c + 1) * BLK]
                        if c < 2:
                            nc.scalar.activation(
                                dst, src, AF.Relu, bias=b1_sb[:, c:c + 1]
                            )
                        else:
                            nc.vector.tensor_scalar(
                                dst, src, b1_sb[:, c:c + 1], 0.0,
                                op0=ALU.add, op1=ALU.max,
                            )

                    # ---- layer 2 (bias via ones-row): h2 = relu(W2' @ h1') --
                    z2 = z2pool.tile([H2CS, H2C * BLK], F32, tag="z2")
                    for mc in range(H2C):
                        for kc in range(H1C):
                            kk = H1CS + 1 if kc == 0 else H1CS
                            nc.tensor.matmul(
                                z2[:, mc * BLK:(mc + 1) * BLK],
                                lhsT=w2_sb[
                                    :kk,
                                    kc * H2 + mc * H2CS: kc * H2 + (mc + 1) * H2CS,
                                ],
                                rhs=h1[:kk, kc * BLK:(kc + 1) * BLK],
                                start=(kc == 0),
                                stop=(kc == H1C - 1),
                            )
                    nc.scalar.activation(
                        h2[:H2CS, :], z2[:], AF.Relu, bias=0.0
                    )

                    # ---- layer 3: h3 = relu(W3' @ h2') (fp16 out) ----
                    # z3 reuses the z1 slot (z1 is fully consumed by now)
                    z3 = z1pool.tile([H3, BLK], F32, tag="z1", name="z3")
                    for kc in range(H3KC):
                        kk = H3 + 1 if kc == 0 else H3
                        nc.tensor.matmul(
                            z3[:],
                            lhsT=w3_sb[:kk, kc * H3:(kc + 1) * H3],
                            rhs=h2[:kk, kc * BLK:(kc + 1) * BLK],
                            start=(kc == 0),
                            stop=(kc == H3KC - 1),
                        )
                    nc.scalar.activation(h3[:H3, :], z3[:], AF.Relu, bias=0.0)

                    # scatter matmuls of the PREVIOUS block fill the PE gap
                    # while ACT finishes h3 = relu(z3) for this block
                    if pend:
                        pa, pb = pend
                        for g in range(GPB):
                            nc.tensor.matmul(
                                e_ps[:],
                                lhsT=pa[:, g],
                                rhs=pb[:, g],
                                start=(scnt == 0),
                                stop=(scnt == n_scatter - 1),
                            )
                            scnt += 1
                        pend = []

                    # ---- layer 4 (transposed, fp16): v[n] = h3'[:,n] . W4' --
                    v_ps = vpool.tile([128, GPB], F32, tag="v")
                    for g in range(GPB):
                        nc.tensor.matmul(
                            v_ps[:, g:g + 1],
                            lhsT=h3[:, g * 128:(g + 1) * 128],
                            rhs=w4_sb[:],
                            start=(g == 0),
                            stop=(g == GPB - 1),
                        )
                    v_sb = vsbp.tile([128, GPB], F32, tag="vsb")
                    nc.scalar.activation(
                        v_sb[:], v_ps[:], AF.Identity, bias=0.0
                    )

                    # ---- scatter-add as matmul ----
                    c0 = b * GPB
                    i3 = iota_sb[:].rearrange("p (g j) -> p g j", g=GPB)
                    qb = (q_sb[:, c0:c0 + GPB].unsqueeze(2)
                          .broadcast_to([128, GPB, 128]))
                    rb = (r_sb[:, c0:c0 + GPB].unsqueeze(2)
                          .broadcast_to([128, GPB, 128]))
                    vb = (v_sb[:, 0:GPB].unsqueeze(2)
                          .broadcast_to([128, GPB, 128]))

                    t1 = abpool.tile([128, GPB, 128], F32, tag="t1")
                    nc.vector.tensor_tensor(t1[:], i3, qb, op=ALU.is_equal)
                    a_sb = abpool.tile([128, GPB, 128], F16, tag="A")
                    nc.vector.tensor_tensor(a_sb[:], t1[:], vb, op=ALU.mult)
                    b_sb = abpool.tile([128, GPB, 128], F16, tag="B")
                    nc.vector.tensor_tensor(b_sb[:], i3, rb, op=ALU.is_equal)

                    pend = (a_sb, b_sb)

                    if debug and t == 0 and b == 0:
                        nc.sync.dma_start(dbg["h1"], h1[:].bitcast(F32))
                        nc.sync.dma_start(dbg["h2"], h2[:].bitcast(F32))
                        nc.sync.dma_start(dbg["h3"], h3[:])
                        nc.sync.dma_start(dbg["v"], v_sb[:])
                        nc.sync.dma_start(
                            dbg["t1"], t1[:].rearrange("p g j -> p (g j)")
                        )
                        nc.sync.dma_start(
                            dbg["A"], a_sb[:].rearrange("p g j -> p (g j)")
                        )
                        nc.sync.dma_start(
                            dbg["B"], b_sb[:].rearrange("p g j -> p (g j)")
                        )

            if pend:
                pa, pb = pend
                for g in range(GPB):
                    nc.tensor.matmul(
                        e_ps[:],
                        lhsT=pa[:, g],
                        rhs=pb[:, g],
                        start=(scnt == 0),
                        stop=(scnt == n_scatter - 1),
                    )
                    scnt += 1
                pend = []

            e_sb = const.tile([128, 128], F32, tag="eout")
            nc.vector.tensor_copy(e_sb[:], e_ps[:])
            nc.sync.dma_start(eout, e_sb[:])

    nc.compile()
    return nc


def prep_shared(W1, b1, W2, b2, W3, b3, W4, b4):
    """Weight/bias layout marshaling (replicated across cores).

    Biases b2/b3/b4 are folded into an extra contraction row of the
    transposed weights (matched by a ones-row in h1/h2/h3 on device).
    """
    f = np.float32
    w1t = np.ascontiguousarray(W1.transpose(0, 2, 1), dtype=f)          # [T,F,H1]

    w2core = (
        W2.transpose(0, 2, 1)
        .reshape(T, H1C, H1CS, H2)
        .transpose(0, 2, 1, 3)
        .reshape(T, H1CS, H1C * H2)
    )
    w2t = np.zeros((T, H1CS + 1, H1C * H2), dtype=f)
    w2t[:, :H1CS, :] = w2core
    w2t[:, H1CS, :H2] = b2                                               # kc==0 only

    w3core = (
        W3.transpose(0, 2, 1)
        .reshape(T, H3KC, H3, H3)
        .transpose(0, 2, 1, 3)
        .reshape(T, H3, H3KC * H3)
    )
    w3t = np.zeros((T, H3 + 1, H3KC * H3), dtype=f)
    w3t[:, :H3, :] = w3core
    w3t[:, H3, :H3] = b3                                                 # kc==0 only

    w4t = np.zeros((T, H3 + 1, 1), dtype=np.float16)
    w4t[:, :H3, 0] = W4.reshape(T, H3).astype(np.float16)
    w4t[:, H3, 0] = b4.reshape(T).astype(np.float16)

    out = {
        "w1t": w1t,
        "w2t": w2t,
        "w3t": w3t,
        "w4t": w4t,
        "b1": np.ascontiguousarray(b1.reshape(T, H1, 1), dtype=f),
        "iota": np.ascontiguousarray(
            np.broadcast_to(np.arange(128, dtype=f), (128, GPB, 128))
            .reshape(128, BLK)
        ),
        "ones32": np.ones((1, H1C * BLK), dtype=f),
        "ones16": np.ones((1, BLK), dtype=np.float16),
    }
    return out


def prep_core(x, ind, core, ns=NSHARD):
    """Per-core shard marshaling: transposed x and split/transposed indices."""
    f = np.float32
    sl = slice(core * ns, (core + 1) * ns)
    X = ns // 128
    xs = x[:, sl, :]
    xT = np.ascontiguousarray(xs.transpose(0, 2, 1), dtype=f)           # [T,F,ns]
    inds = np.asarray(ind[:, sl])
    q = (inds // 128).astype(f)
    r = (inds % 128).astype(f)
    qT = np.ascontiguousarray(q.reshape(T, X, 128).transpose(0, 2, 1))  # [T,128,X]
    rT = np.ascontiguousarray(r.reshape(T, X, 128).transpose(0, 2, 1))
    return {"xT": xT, "qT": qT, "rT": rT}


_CACHE = {}


def _get_program():
    if "nc" not in _CACHE:
        _CACHE["nc"] = build_program()
    return _CACHE["nc"]


def _ensure_ntff_hook():
    """Install the axon NTFF profile hook if the image's antenv lacks it."""
    import sys
    import types

    try:
        from antenv.axon_hooks import get_axon_ntff_profile_hook  # noqa: F401
        return
    except ImportError:
        pass
    try:
        from trn_agent_boot.trn_boot import _ntff_profile_via_ctypes
    except ImportError:
        return
    so = "/opt/axon/libaxon_pjrt.so"
    if not os.path.exists(so):
        return
    hook = _ntff_profile_via_ctypes(so)
    mod = types.ModuleType("antenv.axon_hooks")
    state = {"hook": hook}
    mod.get_axon_ntff_profile_hook = lambda: state["hook"]
    mod.set_axon_ntff_profile_hook = lambda h: state.update(hook=h)
    sys.modules["antenv.axon_hooks"] = mod


def run(inputs, trace=False, trace_kwargs=None):
    """Run the 8-core kernel. Returns (out [M,1] f32, BassKernelResults)."""
    x = np.asarray(inputs["x"], dtype=np.float32)
    ind = np.asarray(inputs["ind"])
    e = np.asarray(inputs["e"], dtype=np.float32)
    shared = prep_shared(
        np.asarray(inputs["W1"]), np.asarray(inputs["b1"]),
        np.asarray(inputs["W2"]), np.asarray(inputs["b2"]),
        np.asarray(inputs["W3"]), np.asarray(inputs["b3"]),
        np.asarray(inputs["W4"]), np.asarray(inputs["b4"]),
    )
    in_maps = []
    for c in range(NCORES):
        m = dict(shared)
        m.update(prep_core(x, ind, c))
        in_maps.append(m)

    nc = _get_program()
    if trace:
        _ensure_ntff_hook()
    res = run_bass_kernel_spmd(
        nc,
        in_maps,
        core_ids=list(range(NCORES)),
        trace=trace,
        **(trace_kwargs or {}),
    )
    acc = e.reshape(-1).astype(np.float64).copy()
    for rm in res.results:
        acc += rm["e_part"].astype(np.float64).reshape(-1)
    out = acc.astype(np.float32).reshape(MOLS, 1)
    return out, res


def kernel(**inputs):
    out, _ = run(inputs, trace=False)
    return out
